# revision 13
# baseline (speedup 1.0000x reference)
"""Trainium2 Bass kernel for CosineSimMatching (grid-sample cost volume + argmax).

Math summary (per batch b, pixel (h, w)):
  cd = coarse_disp/4; 9 disparity hypotheses d sample right_feat bilinearly at
  ix_d = (w - cd)*W/(W-1) + (d-4)/4 - 0.5, iy = h*H/(H-1) - 0.5 (y fixed per row).
  cos_d = <cost_d, left>/ (||cost_d||*||left||), out = coarse + argmax_d - 4.

Decomposition used on device:
  rfy[c, x]   = wy0*right[y0] + wy1*right[y0+1]      (y-interp; row gather on host)
  D_p[r, w]   = <left[:, r, w], rfy[:, r, w+p-10]>   (13 banded dot-product planes, PE)
  N[x], M[x]  = ||rfy[x]||^2, <rfy[x], rfy[x+1]>     (norm fields, PE)
  Per pixel all 9 hypotheses live in a 4-wide window K-1..K+2 (K = floor of the
  center offset), selected from the 13 planes by fused is_equal+mult ops + PE
  contraction. cos_d = dot_d * rsqrt(nc2_d) with dot/nc2 linear/quadratic in the
  x-frac; hypotheses (d, d+4) share frac structure -> processed pair-packed in
  128 partitions. ||left|| drops out of the argmax.

Layouts: the D planes and the selection tiles use a band-split r-major layout
(partition = 64*(r>=4) + 13*(r%4) + p, rows 52-63/116-127 dead) so the
per-bgroup n/m/k replication tiles are built with two 3-dim DMAs each instead
of 13 per-plane DMAs. The product path runs fp16 (PE 1 cycle/row; DVE 2x);
the window-base (k) chain and the B2 cos/argmax stay fp32.

DMA triggers are split between the sync and scalar queues (each DIRECT2D
issue costs ~0.6us of serial queue time).

Sharding: 8 cores = (2 batches) x (4 slices of 64 rows). right_feat rows are
halo-gathered on host per core; all compute on device.
"""

import math
import numpy as np

B, C, H, W = 2, 64, 256, 512
R = 64            # rows per core
NB = 8            # bgroups (8 rows each)
NG = 32           # 2-row groups
NP = 13           # banded planes, shift k = p - 10
XW = W + 12       # rfy x-range: x in [-10, 513], col = x + 10
F32 = np.float32
F16 = np.float16
S_CD = F32(-128.0 / 511.0)   # base'' = coarse*S_CD + wconst[w]
TIE_TOL = 1e-6

_CACHE = {}


# ----------------------------------------------------------------------------
# host-side constant builders
# ----------------------------------------------------------------------------

def _y_weights():
    """Replicate reference y-interp float ops exactly (float32)."""
    y = np.arange(H, dtype=F32)
    gy0 = (y / F32(H - 1) * F32(2.0) - F32(1.0)).astype(F32)
    iy = ((gy0 + F32(1.0)) * F32(H) - F32(1.0)) * F32(0.5)
    y0 = np.floor(iy)
    wy1 = (iy - y0).astype(F32)
    wy0 = (F32(1.0) - wy1).astype(F32)
    return y0.astype(np.int64), wy0, wy1


def _build_consts():
    wconst = np.array([w * 512.0 / 511.0 - w + 9.5 for w in range(W)], dtype=F32)
    wconst128 = np.broadcast_to(wconst, (128, W)).copy()

    # deltaB [128, 127]: ones at (rt*64+c, 63+rt); slicing cols [63-L : 127-L]
    # puts the one at local col L+rt -> stationary for the N/M field matmuls
    deltaB = np.zeros((128, 127), F16)
    deltaB[0:64, 63] = 1.0
    deltaB[64:128, 64] = 1.0

    # deltaB2 [128, 102]: ones at (rt*64+c, 38+13*rt); slice [38-L : 102-L]
    # puts the one at local col L+13*rt -> banded-plane matmuls write the
    # band-split r-major dps rows 26*(gg%2) + 13*rt + p
    deltaB2 = np.zeros((128, 102), F16)
    deltaB2[0:64, 38] = 1.0
    deltaB2[64:128, 51] = 1.0

    # deltaG2 [128, 120]: ones at (64h+13q+p, 56 + 4h + q); slice
    # [56-8u : 120-8u] puts the one at local col 8u + r (r = 4h+q): the G
    # contraction sums the one live p per (r, w) into slot-major gps rows
    deltaG2 = np.zeros((128, 120), F16)
    for h in range(2):
        for q in range(4):
            for p in range(NP):
                deltaG2[64 * h + 13 * q + p, 56 + 4 * h + q] = 1.0

    # smallconsts [128, 8]: cols 0-3 pio_j in band-split r-major layout
    # (value p+1-j; dead rows -99), cols 4-7 dcol (pair d': lo d'-4, hi d')
    small = np.zeros((128, 8), F32)
    small[:, 0:4] = -99.0
    for h in range(2):
        for q in range(4):
            for p in range(NP):
                for j in range(4):
                    small[64 * h + 13 * q + p, j] = p + 1 - j
    for dp in range(4):
        small[0:64, 4 + dp] = dp - 4
        small[64:128, 4 + dp] = dp
    return wconst, wconst128, deltaB, deltaB2, deltaG2, small


def _host_prep(left_feat, right_feat, coarse_disp):
    """Build the 8 per-core input maps."""
    wconst, wconst128, deltaB, deltaB2, deltaG2, small = _build_consts()
    y0, wy0, wy1 = _y_weights()

    left_feat = np.ascontiguousarray(left_feat, dtype=np.float32)
    right_feat = np.ascontiguousarray(right_feat, dtype=np.float32)
    coarse_disp = np.ascontiguousarray(coarse_disp, dtype=np.float32)

    in_maps = []
    for core in range(8):
        b, h0 = core // 4, (core % 4) * R
        leftsl = np.ascontiguousarray(
            left_feat[b][:, h0:h0 + R, :]).astype(F16)                 # [C, R, W]
        coarse = coarse_disp[b][h0:h0 + R, :]                          # [R, W]

        y_lo = h0 - 1 if h0 < 128 else h0
        rightp = np.zeros((R + 1, C, XW), F16)
        lo = max(0, y_lo)
        hi = min(H, y_lo + R + 1)
        rightp[lo - y_lo:hi - y_lo, :, 10:10 + W] = (
            right_feat[b][:, lo:hi, :].transpose(1, 0, 2).astype(F16))

        # wys [128, 64]: cols 0:32 wy0 per group, 32:64 wy1; partition rt*64+c
        wys = np.zeros((128, 64), F32)
        for g in range(NG):
            for rt in range(2):
                hh = h0 + 2 * g + rt
                wys[rt * 64:(rt + 1) * 64, g] = wy0[hh]
                wys[rt * 64:(rt + 1) * 64, 32 + g] = wy1[hh]

        cd128 = np.concatenate([coarse, coarse], axis=0).astype(F32)   # [128, W]

        in_maps.append({
            "leftsl": leftsl,
            "rightp": rightp,
            "wys": wys,
            "cd128": cd128,
            "wconst128": wconst128,
            "deltaB": deltaB,
            "deltaB2": deltaB2,
            "deltaG2": deltaG2,
            "small": small,
        })
    return in_maps


# ----------------------------------------------------------------------------
# bass kernel
# ----------------------------------------------------------------------------

def _build_bass():
    import concourse.bacc as bacc
    import concourse.tile as tile
    from concourse import mybir

    f32 = mybir.dt.float32
    f16 = mybir.dt.float16
    Alu = mybir.AluOpType
    Act = mybir.ActivationFunctionType

    nc = bacc.Bacc("TRN2")

    t_left = nc.dram_tensor("leftsl", [C, R, W], f16, kind="ExternalInput")
    t_right = nc.dram_tensor("rightp", [R + 1, C, XW], f16, kind="ExternalInput")
    t_wys = nc.dram_tensor("wys", [128, 64], f32, kind="ExternalInput")
    t_cd128 = nc.dram_tensor("cd128", [128, W], f32, kind="ExternalInput")
    t_wc128 = nc.dram_tensor("wconst128", [128, W], f32, kind="ExternalInput")
    t_dB = nc.dram_tensor("deltaB", [128, 127], f16, kind="ExternalInput")
    t_dB2 = nc.dram_tensor("deltaB2", [128, 102], f16, kind="ExternalInput")
    t_dG2 = nc.dram_tensor("deltaG2", [128, 120], f16, kind="ExternalInput")
    t_small = nc.dram_tensor("small", [128, 8], f32, kind="ExternalInput")
    t_out = nc.dram_tensor("out", [R, W], f32, kind="ExternalOutput")

    with tile.TileContext(nc) as tc:
        with (
            tc.tile_pool(name="singles", bufs=1) as singles,
            tc.tile_pool(name="persist", bufs=1) as persist,
            tc.tile_pool(name="ldpool", bufs=3) as ldpool,
            tc.tile_pool(name="rfpool", bufs=3) as rfpool,
            tc.tile_pool(name="prodpool", bufs=6) as prodpool,
            tc.tile_pool(name="sqpool", bufs=3) as sqpool,
            tc.tile_pool(name="fieldpool", bufs=2) as fieldpool,
            tc.tile_pool(name="n104pool", bufs=2) as n104pool,
            tc.tile_pool(name="prdsel_p", bufs=7) as prdsel_p,
            tc.tile_pool(name="b2pool", bufs=10) as b2pool,
            tc.tile_pool(name="mupool", bufs=4) as mupool,
            tc.tile_pool(name="sel8pool", bufs=4) as sel8pool,
            tc.tile_pool(name="dpsums", bufs=2, space="PSUM") as dpsums,
            tc.tile_pool(name="fpsums", bufs=2, space="PSUM") as fpsums,
            tc.tile_pool(name="gpsums", bufs=2, space="PSUM") as gpsums,
        ):
            # ---- constants to SBUF ----
            wys_sb = singles.tile([128, 64], f32, name="wys_sb")
            nc.sync.dma_start(out=wys_sb, in_=t_wys[:])
            wc128_sb = singles.tile([128, W], f32, name="wc128_sb")
            nc.sync.dma_start(out=wc128_sb, in_=t_wc128[:])
            dB_sb = singles.tile([128, 127], f16, name="dB_sb")
            nc.sync.dma_start(out=dB_sb, in_=t_dB[:])
            dB2_sb = singles.tile([128, 102], f16, name="dB2_sb")
            nc.sync.dma_start(out=dB2_sb, in_=t_dB2[:])
            dG2_sb = singles.tile([128, 120], f16, name="dG2_sb")
            nc.sync.dma_start(out=dG2_sb, in_=t_dG2[:])
            small_sb = singles.tile([128, 8], f32, name="small_sb")
            nc.sync.dma_start(out=small_sb, in_=t_small[:])
            cd128_sb = singles.tile([128, W], f32, name="cd128_sb")
            nc.sync.dma_start(out=cd128_sb, in_=t_cd128[:])

            # ---- once: window-base chain (fp32 exact floor) ----
            base128 = persist.tile([128, W], f32, name="base128")
            nc.vector.scalar_tensor_tensor(
                base128, cd128_sb, float(S_CD), wc128_sb, Alu.mult, Alu.add)
            bm128 = b2pool.tile([128, W], f32, name="bm128", tag="scrA")
            nc.vector.tensor_scalar_sub(bm128, base128, 0.5)
            ki128 = mupool.tile([128, W], mybir.dt.int32, name="ki128", tag="ki32")
            nc.vector.tensor_copy(ki128, bm128)
            kf128 = persist.tile([128, W], f32, name="kf128")
            nc.vector.tensor_copy(kf128, ki128)
            t128 = persist.tile([128, W], f32, name="t128")
            nc.vector.tensor_sub(t128, base128, kf128)
            kf16 = persist.tile([128, W], f16, name="kf16")
            nc.vector.tensor_copy(kf16, ki128)

            # pairs[:, i, :]: 0 GP01, 1 GP12, 2 GP23, 3 NP01, 4 NP12, 5 NP23,
            # 6 MP01, 7 MP12;  partition = 64*half + 8*bg + r
            pairs = persist.tile([128, 8, W], f32, name="pairs")
            # base-0 copies of G3 / N3 / M2 rows for the d=8 step
            d8pl = persist.tile([64, 3, W], f32, name="d8pl")

            # 11 plane slots in the per-bgroup G psum: G0-3, N0-3, M0-2
            slot_src = [
                (0, "D", 0), (1, "D", 1), (2, "D", 2), (3, "D", 3),
                (4, "N", 0), (5, "N", 1), (6, "N", 2), (7, "N", 3),
                (8, "M", 0), (9, "M", 1), (10, "M", 2),
            ]
            # pair tensor pi gets (lo_slot, hi_slot)
            pair_slots = [(0, 1), (1, 2), (2, 3), (4, 5), (5, 6), (6, 7),
                          (8, 9), (9, 10)]

            for bg in range(NB):
                dps = dpsums.tile([128, W], f32, name="dps", tag="dps")
                f1n = fpsums.tile([64, W], f32, name="f1n", tag="f1n")
                # f1m at [0:64]; N-tail at [64:96, 0:12]; M-tail [64:96, 12:23]
                fall2 = fpsums.tile([128, W], f32, name="fall2", tag="fall2")

                # ---------------- stage A: 4 groups of 2 rows ----------------
                for gg in range(4):
                    g = 4 * bg + gg
                    left2 = ldpool.tile([128, W], f16, name="left2", tag="left2")
                    nc.sync.dma_start(
                        out=left2,
                        in_=t_left[:, 2 * g:2 * g + 2, :].rearrange("c r w -> r c w"),
                    )
                    ra2 = ldpool.tile([128, XW], f16, name="ra2", tag="ra2")
                    nc.sync.dma_start(
                        out=ra2,
                        in_=t_right[2 * g:2 * g + 2],
                    )
                    rb2 = ldpool.tile([128, XW], f16, name="rb2", tag="rb2")
                    nc.sync.dma_start(
                        out=rb2,
                        in_=t_right[2 * g + 1:2 * g + 3],
                    )

                    rfy2 = rfpool.tile([128, XW], f16, name="rfy2", tag="rfy2")
                    tmpw = rfpool.tile([128, XW], f16, name="tmpw", tag="tmpw")
                    tmpv = rfpool.tile([128, XW], f16, name="tmpv", tag="tmpv")
                    # both scaled copies on the Scalar engine; the plain add is
                    # 2x-eligible on DVE (the fused STT form is not)
                    nc.scalar.activation(tmpw, rb2, Act.Copy,
                                         scale=wys_sb[:, 32 + g:33 + g])
                    nc.scalar.activation(tmpv, ra2, Act.Copy,
                                         scale=wys_sb[:, g:g + 1])
                    nc.vector.tensor_add(rfy2, tmpv, tmpw)

                    nsq2 = sqpool.tile([128, XW], f16, name="nsq2", tag="nsq2")
                    nc.scalar.activation(nsq2, rfy2, Act.Square)
                    mpr2 = sqpool.tile([128, XW - 1], f16, name="mpr2", tag="mpr2")
                    nc.vector.tensor_mul(mpr2, rfy2[:, 0:XW - 1], rfy2[:, 1:XW])

                    # field matmuls (N/M mains + tails)
                    nc.tensor.matmul(
                        f1n[0:64, :], dB_sb[:, 63 - 2 * gg:127 - 2 * gg], nsq2[:, 0:W],
                        start=(gg == 0), stop=(gg == 3), skip_group_check=True)
                    nc.tensor.matmul(
                        fall2[0:64, :], dB_sb[:, 63 - 2 * gg:127 - 2 * gg], mpr2[:, 0:W],
                        start=(gg == 0), stop=(gg == 3), skip_group_check=True)
                    nc.tensor.matmul(
                        fall2[64:96, 0:12], dB_sb[:, 63 - 2 * gg:95 - 2 * gg],
                        nsq2[:, W:XW],
                        start=(gg == 0), stop=(gg == 3), skip_group_check=True)
                    nc.tensor.matmul(
                        fall2[64:96, 12:23], dB_sb[:, 63 - 2 * gg:95 - 2 * gg],
                        mpr2[:, W:XW - 1],
                        start=(gg == 0), stop=(gg == 3), skip_group_check=True)

                    # 13 banded products + delta matmuls writing the band-split
                    # r-major dps rows 64*(gg>=2) + 26*(gg%2) + 13*rt + p
                    for p in range(NP):
                        prod = prodpool.tile([128, W], f16, name="prod", tag="prod")
                        eng = nc.gpsimd if p in (3, 6, 12) else nc.vector
                        eng.tensor_mul(prod, left2, rfy2[:, p:p + W])
                        base = 0 if gg < 2 else 64
                        lcol = 26 * (gg % 2) + p
                        nc.tensor.matmul(
                            dps[base:base + 64, :], dB2_sb[:, 38 - lcol:102 - lcol],
                            prod,
                            start=(p == 0 and gg in (0, 2)),
                            stop=(p == 12 and gg in (1, 3)),
                            skip_group_check=True)

                # ---------------- stage A2: fields -> sbuf -> replicated ----
                nf = fieldpool.tile([8, XW], f16, name="nf", tag="nf")
                mf = fieldpool.tile([8, XW], f16, name="mf", tag="mf")
                tails = fieldpool.tile([128, 24], f16, name="tails", tag="tails")
                nc.scalar.copy(nf[0:8, 0:W], f1n[0:8, 0:W])
                nc.scalar.copy(mf[0:8, 0:W], fall2[0:8, 0:W])
                nc.scalar.copy(tails[64:72, 0:23], fall2[64:72, 0:23])
                nc.vector.memset(mf[0:8, XW - 1:XW], 0.0)
                nc.scalar.dma_start(out=nf[0:8, W:XW], in_=tails[64:72, 0:12])
                nc.scalar.dma_start(out=mf[0:8, W:XW - 1], in_=tails[64:72, 12:23])

                # n/m/k in band-split r-major layout: partition 64h+13q+p,
                # one 3-dim DMA per 52-row band (p is a col-offset dim on src)
                n104 = n104pool.tile([128, W], f16, name="n104", tag="n104")
                m104 = n104pool.tile([128, W], f16, name="m104", tag="m104")
                k104 = n104pool.tile([128, W], f16, name="k104", tag="k104")
                if bg < 2:
                    # pool bufs=2: zero each physical buffer once before the
                    # band DMAs fill the live rows, so the dead rows 52-63 /
                    # 116-127 stay finite (0 * anything) in the fused selects
                    nc.vector.memset(n104, 0.0)
                    nc.vector.memset(m104, 0.0)
                from concourse.ap import AP as _AP

                def _win(base, pstride):
                    # [4, 13, W]: dims (row q, plane p, col w); src col = p*pstride + w
                    return _AP(base.tensor, base.offset,
                               [list(base.ap[0]), [pstride, NP], [1, W]])

                for h in range(2):
                    nc.sync.dma_start(
                        out=n104[64 * h:64 * h + 52, :],
                        in_=_win(nf[4 * h:4 * h + 4, 0:W], 1))
                    nc.sync.dma_start(
                        out=m104[64 * h:64 * h + 52, :],
                        in_=_win(mf[4 * h:4 * h + 4, 0:W], 1))
                    nc.sync.dma_start(
                        out=k104[64 * h:64 * h + 52, :],
                        in_=_win(kf16[8 * bg + 4 * h:8 * bg + 4 * h + 4, :], 0))

                # fp16 copy of the banded-plane psum for the fused selects
                dps16 = prdsel_p.tile([128, W], f16, name="dps16", tag="dps16")
                nc.scalar.copy(dps16, dps)

                # one-hot masks (tensor_scalar is 2x-eligible in fp16; the
                # fused is_eq+mult scalar_tensor_tensor is not), then plain
                # 2x multiplies for the plane selection
                masks = []
                for j in range(4):
                    mk = prdsel_p.tile([128, W], f16, name=f"mk{j}", tag="mk")
                    nc.vector.tensor_scalar(
                        mk, k104, small_sb[:, j:j + 1], None, Alu.is_equal)
                    masks.append(mk)
                prd = {}
                for j in range(4):
                    tj = prdsel_p.tile([128, W], f16, name=f"prd{j}", tag="prdsel")
                    nc.vector.tensor_mul(tj, masks[j], dps16)
                    prd[("D", j)] = tj
                for j in range(4):
                    tj = prdsel_p.tile([128, W], f16, name=f"prn{j}", tag="prdsel")
                    nc.vector.tensor_mul(tj, masks[j], n104)
                    prd[("N", j)] = tj
                for j in range(3):
                    tj = prdsel_p.tile([128, W], f16, name=f"prm{j}", tag="prdsel")
                    nc.vector.tensor_mul(tj, masks[j], m104)
                    prd[("M", j)] = tj

                gps = gpsums.tile([128, W], f32, name="gps", tag="gps")
                for s, kind, j in slot_src:
                    base = 0 if s < 8 else 64
                    u = s % 8
                    nc.tensor.matmul(
                        gps[base:base + 64, :], dG2_sb[:, 56 - 8 * u:120 - 8 * u],
                        prd[(kind, j)],
                        start=(s in (0, 8)), stop=(s in (7, 10)), skip_group_check=True)

                gsb = b2pool.tile([128, W], f32, name="gsb", tag="scrA")
                nc.scalar.copy(gsb, gps)
                for pi, (slo, shi) in enumerate(pair_slots):
                    nc.scalar.dma_start(
                        out=pairs[8 * bg:8 * bg + 8, pi, :],
                        in_=gsb[8 * slo:8 * slo + 8, :])
                    nc.scalar.dma_start(
                        out=pairs[64 + 8 * bg:64 + 8 * bg + 8, pi, :],
                        in_=gsb[8 * shi:8 * shi + 8, :])
                for k, s in ((0, 3), (1, 7), (2, 10)):
                    nc.scalar.dma_start(
                        out=d8pl[8 * bg:8 * bg + 8, k, :],
                        in_=gsb[8 * s:8 * s + 8, :])

            # ---------------- stage B2: pair interpolation + argmax ---------
            sg01 = persist.tile([128, W], f32, name="sg01")
            sg12 = persist.tile([128, W], f32, name="sg12")
            ta01 = persist.tile([128, W], f32, name="ta01")
            ta12 = persist.tile([128, W], f32, name="ta12")
            u1 = b2pool.tile([128, W], f32, name="u1", tag="scrA")
            nc.vector.tensor_add(u1, pairs[:, 3, :], pairs[:, 4, :])
            nc.vector.scalar_tensor_tensor(sg01, pairs[:, 6, :], -2.0, u1, Alu.mult, Alu.add)
            u2 = b2pool.tile([128, W], f32, name="u2", tag="scrA")
            nc.vector.tensor_add(u2, pairs[:, 4, :], pairs[:, 5, :])
            nc.vector.scalar_tensor_tensor(sg12, pairs[:, 7, :], -2.0, u2, Alu.mult, Alu.add)
            nc.vector.tensor_sub(ta01, pairs[:, 6, :], pairs[:, 3, :])
            nc.vector.tensor_sub(ta12, pairs[:, 7, :], pairs[:, 4, :])

            best = persist.tile([128, W], f32, name="best")
            outp = persist.tile([128, W], f32, name="outp")

            for dp in range(4):
                if dp == 0:
                    fpl = t128
                    GA, GB = pairs[:, 0, :], pairs[:, 1, :]
                    NA, SG, TA = pairs[:, 3, :], sg01, ta01
                else:
                    sel = b2pool.tile([128, W], f32, name="sel", tag="scrA")
                    nc.vector.tensor_scalar(sel, t128, float(1.0 - 0.25 * dp), None, Alu.is_ge)
                    fpl = b2pool.tile([128, W], f32, name="fpl", tag="scrA")
                    nc.vector.scalar_tensor_tensor(
                        fpl, t128, float(0.25 * dp), sel, Alu.add, Alu.subtract)
                    sel8 = sel8pool.tile([128, W], mybir.dt.int8, name="sel8", tag="sel8")
                    nc.vector.tensor_copy(sel8, sel)
                    GA = b2pool.tile([128, W], f32, name="GA", tag="scrA")
                    nc.scalar.copy(GA, pairs[:, 0, :])
                    nc.vector.copy_predicated(GA, sel8, pairs[:, 1, :])
                    GB = b2pool.tile([128, W], f32, name="GB", tag="scrA")
                    nc.scalar.copy(GB, pairs[:, 1, :])
                    nc.vector.copy_predicated(GB, sel8, pairs[:, 2, :])
                    NA = b2pool.tile([128, W], f32, name="NA", tag="scrA")
                    nc.scalar.copy(NA, pairs[:, 3, :])
                    nc.vector.copy_predicated(NA, sel8, pairs[:, 4, :])
                    SG = b2pool.tile([128, W], f32, name="SG", tag="scrA")
                    nc.scalar.copy(SG, sg01)
                    nc.vector.copy_predicated(SG, sel8, sg12)
                    TA = b2pool.tile([128, W], f32, name="TA", tag="scrA")
                    nc.scalar.copy(TA, ta01)
                    nc.vector.copy_predicated(TA, sel8, ta12)

                diff = b2pool.tile([128, W], f32, name="diff", tag="scrA")
                nc.vector.tensor_sub(diff, GB, GA)
                qd = b2pool.tile([128, W], f32, name="qd", tag="scrA")
                nc.vector.tensor_mul(qd, fpl, diff)
                dot = b2pool.tile([128, W], f32, name="dot", tag="scrA")
                nc.vector.tensor_add(dot, qd, GA)

                q1 = b2pool.tile([128, W], f32, name="q1", tag="scrA")
                nc.vector.tensor_mul(q1, SG, fpl)
                q2 = b2pool.tile([128, W], f32, name="q2", tag="scrA")
                nc.vector.scalar_tensor_tensor(q2, TA, 2.0, q1, Alu.mult, Alu.add)
                q3 = b2pool.tile([128, W], f32, name="q3", tag="scrA")
                nc.vector.tensor_mul(q3, fpl, q2)
                nc2t = b2pool.tile([128, W], f32, name="nc2t", tag="scrA")
                nc.vector.tensor_add(nc2t, q3, NA)

                nc.vector.tensor_scalar_max(nc2t, nc2t, 1e-30)
                den = b2pool.tile([128, W], f32, name="den", tag="scrA")
                nc.scalar.activation(den, nc2t, Act.Sqrt)
                inv = b2pool.tile([128, W], f32, name="inv", tag="scrA")
                rscr = b2pool.tile([128, W], f32, name="rscr", tag="scrA")
                nc.vector.reciprocal_approx_accurate(inv, den, rscr)
                cos = b2pool.tile([128, W], f32, name="cos", tag="scrA")
                nc.vector.tensor_mul(cos, dot, inv)

                cand = b2pool.tile([128, W], f32, name="cand", tag="scrA")
                nc.vector.tensor_scalar(
                    cand, cd128_sb, small_sb[:, 4 + dp:5 + dp], None, Alu.add)

                if dp == 0:
                    nc.scalar.copy(best, cos)
                    nc.scalar.copy(outp, cand)
                else:
                    mu = mupool.tile([128, W], mybir.dt.int8, name="mu", tag="mu8")
                    nc.vector.scalar_tensor_tensor(
                        mu, best, TIE_TOL, cos, Alu.add, Alu.is_lt)
                    nc.vector.copy_predicated(best, mu, cos)
                    nc.vector.copy_predicated(outp, mu, cand)

            # merge halves (hi = d in 4..7 wins only if strictly better);
            # hi halves staged to base-0 via sbuf-sbuf DMA (walrus requires
            # equal start partitions on compute operands)
            bh0 = b2pool.tile([64, W], f32, name="bh0", tag="scrA")
            nc.scalar.dma_start(out=bh0, in_=best[64:128, :])
            oh0 = b2pool.tile([64, W], f32, name="oh0", tag="scrA")
            nc.scalar.dma_start(out=oh0, in_=outp[64:128, :])
            m2 = mupool.tile([64, W], mybir.dt.int8, name="m2", tag="mu8")
            nc.vector.scalar_tensor_tensor(
                m2, best[0:64, :], TIE_TOL, bh0, Alu.add, Alu.is_lt)
            nc.vector.copy_predicated(best[0:64, :], m2, bh0)
            nc.vector.copy_predicated(outp[0:64, :], m2, oh0)

            # d = 8: u = t + 2, a = 2, f = t
            f8 = t128[0:64, :]
            GA8, GB8 = pairs[0:64, 2, :], d8pl[:, 0, :]
            NA8, NB8 = pairs[0:64, 5, :], d8pl[:, 1, :]
            MA8 = d8pl[:, 2, :]
            u8 = b2pool.tile([64, W], f32, name="u8", tag="scrA")
            nc.vector.tensor_add(u8, NA8, NB8)
            sg8 = b2pool.tile([64, W], f32, name="sg8", tag="scrA")
            nc.vector.scalar_tensor_tensor(sg8, MA8, -2.0, u8, Alu.mult, Alu.add)
            ta8 = b2pool.tile([64, W], f32, name="ta8", tag="scrA")
            nc.vector.tensor_sub(ta8, MA8, NA8)
            diff8 = b2pool.tile([64, W], f32, name="diff8", tag="scrA")
            nc.vector.tensor_sub(diff8, GB8, GA8)
            qd8 = b2pool.tile([64, W], f32, name="qd8", tag="scrA")
            nc.vector.tensor_mul(qd8, f8, diff8)
            dot8 = b2pool.tile([64, W], f32, name="dot8", tag="scrA")
            nc.vector.tensor_add(dot8, qd8, GA8)
            q18 = b2pool.tile([64, W], f32, name="q18", tag="scrA")
            nc.vector.tensor_mul(q18, sg8, f8)
            q28 = b2pool.tile([64, W], f32, name="q28", tag="scrA")
            nc.vector.scalar_tensor_tensor(q28, ta8, 2.0, q18, Alu.mult, Alu.add)
            q38 = b2pool.tile([64, W], f32, name="q38", tag="scrA")
            nc.vector.tensor_mul(q38, f8, q28)
            nc28 = b2pool.tile([64, W], f32, name="nc28", tag="scrA")
            nc.vector.tensor_add(nc28, q38, NA8)
            nc.vector.tensor_scalar_max(nc28, nc28, 1e-30)
            den8 = b2pool.tile([64, W], f32, name="den8", tag="scrA")
            nc.scalar.activation(den8, nc28, Act.Sqrt)
            inv8 = b2pool.tile([64, W], f32, name="inv8", tag="scrA")
            rscr8 = b2pool.tile([64, W], f32, name="rscr8", tag="scrA")
            nc.vector.reciprocal_approx_accurate(inv8, den8, rscr8)
            cos8 = b2pool.tile([64, W], f32, name="cos8", tag="scrA")
            nc.vector.tensor_mul(cos8, dot8, inv8)
            cand8 = b2pool.tile([64, W], f32, name="cand8", tag="scrA")
            nc.vector.tensor_scalar_add(cand8, cd128_sb[0:64, :], 4.0)
            m8 = mupool.tile([64, W], mybir.dt.int8, name="m8", tag="mu8")
            nc.vector.scalar_tensor_tensor(
                m8, best[0:64, :], TIE_TOL, cos8, Alu.add, Alu.is_lt)
            nc.vector.copy_predicated(outp[0:64, :], m8, cand8)

            nc.sync.dma_start(out=t_out[:], in_=outp[0:64, :])

    nc.finalize()
    return nc


def _get_nc():
    if "nc" not in _CACHE:
        _CACHE["nc"] = _build_bass()
    return _CACHE["nc"]


def kernel(left_feat, right_feat, coarse_disp):
    from concourse.bass_utils import run_bass_kernel_spmd

    nc = _get_nc()
    in_maps = _host_prep(np.asarray(left_feat), np.asarray(right_feat),
                         np.asarray(coarse_disp))
    res = run_bass_kernel_spmd(nc, in_maps, core_ids=list(range(8)))
    out = np.zeros((B, 1, H, W), np.float32)
    for core in range(8):
        b, h0 = core // 4, (core % 4) * R
        out[b, 0, h0:h0 + R, :] = res.results[core]["out"]
    return out


def run_profiled(left_feat, right_feat, coarse_disp):
    """Run once with NTFF tracing; returns max-core exec time in ns (or None)."""
    from concourse.bass_utils import run_bass_kernel_spmd

    nc = _get_nc()
    in_maps = _host_prep(np.asarray(left_feat), np.asarray(right_feat),
                         np.asarray(coarse_disp))
    res = run_bass_kernel_spmd(nc, in_maps, core_ids=list(range(8)), trace=True)
    if res.instructions_and_trace is not None:
        print(f"trace: {res.instructions_and_trace[1]}")
    return res.exec_time_ns
